# revision 25
# baseline (speedup 1.0000x reference)
"""Paged decode attention (GQA) on 8 trn2 NeuronCores.

Strategy (data parallel over sequences):
  - Host bin-packs the 32 sequences onto 8 cores (4 slots/core, LPT on valid
    block count) and builds, per core, a flat list of 4-token "chunks" to
    gather (only valid blocks -> ~2x traffic saving vs dense).
  - The new-token K/V (k_new/v_new) is handled host-side: the 4-token chunk
    containing position len-1 is redirected to a small per-sequence "patch"
    row appended to the cache, so the device never scatters into the cache.
  - Device per iteration (512 tokens): dma_gather K/V chunks -> PE transposes
    K per head -> scores matmul (float32r, full-rate fp32) -> additive mask
    (+0 / -1e30, ships row-ownership + padding) -> exp with row-sum accum
    (fixed-max softmax; exact after normalization) -> PE transpose of probs ->
    PV matmul accumulated in PSUM across all iterations -> final 1/sum scale.
"""

import math

import numpy as np

B = 32
H = 32
KVH = 8
G = 4
DH = 128
BS = 16
NBLK = 128
NUM_BLOCKS = B * NBLK
SCALE = DH ** -0.5

NCORES = 8
SLOTS = 4           # sequences per core
CHUNK = 4           # tokens per gathered row
ROWF = KVH * DH     # 1024 floats per token
ELEM = CHUNK * ROWF  # 4096 floats per chunk row
NCH_CACHE = NUM_BLOCKS * BS // CHUNK   # 16384 chunks in the cache
GPB = BS // CHUNK   # chunk groups per block = 4
NEG = -1.0e30


def _schedule(lens):
    """LPT bin-packing of sequences onto cores, 4 slots each."""
    nch = [(l + CHUNK - 1) // CHUNK for l in lens]
    order = sorted(range(B), key=lambda s: -nch[s])
    loads = [0] * NCORES
    counts = [0] * NCORES
    assign = [[] for _ in range(NCORES)]
    for s in order:
        c = min(
            (c for c in range(NCORES) if counts[c] < SLOTS),
            key=lambda c: loads[c],
        )
        assign[c].append(s)
        loads[c] += nch[s]
        counts[c] += 1
    t_iter = max(1, max((l + 127) // 128 for l in loads))
    return assign, nch, t_iter


def _host_prepare(q, k_new, v_new, k_cache, v_cache, block_tables, context_lens):
    lens = [int(x) for x in context_lens]
    bt = np.asarray(block_tables)
    assign, nch, T = _schedule(lens)

    kc_flat = np.ascontiguousarray(k_cache).reshape(NUM_BLOCKS * BS, ROWF)
    vc_flat = np.ascontiguousarray(v_cache).reshape(NUM_BLOCKS * BS, ROWF)
    kn = np.ascontiguousarray(k_new).reshape(B, ROWF)
    vn = np.ascontiguousarray(v_new).reshape(B, ROWF)

    # patch rows: the 4-token group holding position len-1, with that token's
    # row replaced by k_new/v_new
    kpatch = np.zeros((B, ELEM), np.float32)
    vpatch = np.zeros((B, ELEM), np.float32)
    for s in range(B):
        l = lens[s]
        g = (l - 1) // CHUNK
        blk = int(bt[s, g // GPB])
        base_slot = blk * BS + (g % GPB) * CHUNK
        krows = kc_flat[base_slot : base_slot + CHUNK].copy()
        vrows = vc_flat[base_slot : base_slot + CHUNK].copy()
        krows[(l - 1) % CHUNK] = kn[s]
        vrows[(l - 1) % CHUNK] = vn[s]
        kpatch[s] = krows.reshape(-1)
        vpatch[s] = vrows.reshape(-1)
    kc4 = np.concatenate([kc_flat.reshape(NCH_CACHE, ELEM), kpatch], axis=0)
    vc4 = np.concatenate([vc_flat.reshape(NCH_CACHE, ELEM), vpatch], axis=0)

    qs = np.asarray(q, np.float32)
    per_core = []
    for c in range(NCORES):
        seqs = assign[c]
        n = T * 128
        cid = np.zeros(n, np.int64)          # chunk ids
        cslot = np.full(n, -1, np.int64)     # owning slot, -1 = padding
        cbase = np.zeros(n, np.int64)        # first token index of chunk
        clen = np.zeros(n, np.int64)         # owning seq len
        pos = 0
        for slot, s in enumerate(seqs):
            l = lens[s]
            ns = nch[s]
            gpatch = (l - 1) // CHUNK
            g = np.arange(ns)
            ids = bt[s, g // GPB].astype(np.int64) * GPB + g % GPB
            ids[gpatch] = NCH_CACHE + s
            cid[pos : pos + ns] = ids
            cslot[pos : pos + ns] = slot
            cbase[pos : pos + ns] = g * CHUNK
            clen[pos : pos + ns] = l
            pos += ns

        # gather index tensor [128, T*8] int16; index j of iter t lives at
        # [j % 16, t*8 + j//16], replicated across the 8 16-partition groups
        idx = np.zeros((128, T * 8), np.int16)
        for t in range(T):
            blk_ids = cid[t * 128 : (t + 1) * 128]
            tile16 = blk_ids.reshape(8, 16).T.astype(np.int16)  # [16, 8]
            idx[:, t * 8 : (t + 1) * 8] = np.tile(tile16, (8, 1))

        # additive mask [16 rows (s,g), T*512]; col (t, j, p) <-> token j of
        # the chunk at position t*128+p; identical for all kv heads
        row_slot = np.arange(16) // 4                             # [16]
        mask = np.full((16, T * 512), NEG, np.float32)
        for t in range(T):
            sl = cslot[t * 128 : (t + 1) * 128]                   # [128]
            tb = cbase[t * 128 : (t + 1) * 128]
            ln = clen[t * 128 : (t + 1) * 128]
            j = np.arange(CHUNK)[:, None]                         # [4,1]
            valid = (tb[None, :] + j < ln[None, :]) & (sl[None, :] >= 0)
            ok = (row_slot[:, None, None] == sl[None, None, :]) & valid[None]
            m = np.where(ok, 0.0, NEG).astype(np.float32)         # [16,4,128]
            mask[:, t * 512 : (t + 1) * 512] = m.reshape(16, 512)

        # qT [128 d, 128 rows], pre-scaled
        qm = np.zeros((128, 128), np.float32)
        for slot, s in enumerate(seqs):
            # rows k*16 + slot*4 + g  <-  q[s, k*4+g, :] * SCALE
            qr = qs[s].reshape(KVH, G, DH) * SCALE                # [8,4,128]
            qm.reshape(KVH, SLOTS, G, 128)[:, slot] = qr
        qt = np.ascontiguousarray(qm.T)

        per_core.append(
            dict(idx=idx, mask=mask, qt=qt, seqs=seqs)
        )
    return kc4, vc4, per_core, T, assign


# ---------------------------------------------------------------------------
# device program
# ---------------------------------------------------------------------------

def _build_program(T, use_f32r=True, repeat=1):
    import concourse.bass as bass  # noqa: F401
    import concourse.mybir as mybir
    import concourse.tile as tile
    from concourse import bacc
    from concourse.masks import make_identity

    f32 = mybir.dt.float32
    f32r = mybir.dt.float32r
    i16 = mybir.dt.int16
    Alu = mybir.AluOpType
    Act = mybir.ActivationFunctionType

    # float32r: full-rate fp32 matmul (N>=256). Compute producers write
    # f32r-typed tiles (walrus inserts the rounding); DMA-fed operands are
    # bitcast at the consumer.
    mmdt = f32r if use_f32r else f32

    def rcv(ap):  # bitcast a DMA-produced operand for the matmul
        return ap.bitcast(f32r) if use_f32r else ap

    nc = bacc.Bacc(
        "TRN2", target_bir_lowering=False, debug=False, num_devices=NCORES
    )
    kc_d = nc.dram_tensor("kc4", [NCH_CACHE + B, ELEM], f32, kind="ExternalInput")
    vc_d = nc.dram_tensor("vc4", [NCH_CACHE + B, ELEM], f32, kind="ExternalInput")
    qt_d = nc.dram_tensor("qt", [128, 128], f32, kind="ExternalInput")
    idx_d = nc.dram_tensor("idx", [128, T * 8], i16, kind="ExternalInput")
    mask_d = nc.dram_tensor("mask", [16, T * 512], f32, kind="ExternalInput")
    out_d = nc.dram_tensor("o", [128, 128], f32, kind="ExternalOutput")

    with tile.TileContext(nc) as tc:
        with (
            tc.tile_pool(name="const", bufs=1) as constp,
            tc.tile_pool(name="kbuf", bufs=2) as kpool,
            tc.tile_pool(name="vbuf", bufs=2) as vpool,
            tc.tile_pool(name="ktbuf", bufs=2) as ktpool,
            tc.tile_pool(name="sco", bufs=2) as spool,
            tc.tile_pool(name="prb", bufs=2) as ppool,
            tc.tile_pool(name="prt", bufs=2) as ptpool,
            tc.tile_pool(name="msk", bufs=2) as mpool,
            tc.tile_pool(name="ktp", bufs=2, space="PSUM") as ktpsum,
            tc.tile_pool(name="scp", bufs=2, space="PSUM") as spsum,
            tc.tile_pool(name="ptp", bufs=2, space="PSUM") as ptpsum,
            tc.tile_pool(name="pvp", bufs=1, space="PSUM") as pvpool,
        ):
            ident = constp.tile([128, 128], f32)
            make_identity(nc, ident[:])
            qt = constp.tile([128, 128], f32)
            nc.sync.dma_start(qt[:], qt_d.ap())
            if use_f32r:
                qt_r = constp.tile([128, 128], f32r)
                nc.vector.tensor_copy(qt_r[:], qt[:])
                ident_r = constp.tile([128, 128], f32r)
                nc.vector.tensor_copy(ident_r[:], ident[:])
            else:
                qt_r = qt
                ident_r = ident
            idxs = constp.tile([128, T * 8], i16)
            nc.sync.dma_start(idxs[:], idx_d.ap())
            # per-(iter, head) row sums of probs; col = t*8 + k
            partials = constp.tile([16, T * 8], f32)

            pv = [
                pvpool.tile([64, 512], f32, name=f"pv{g2}", tag=f"pv{g2}")
                for g2 in range(2)
            ]

            for rep in range(repeat):
              for t in range(T):
                k_tile = kpool.tile([128, ELEM], mmdt)
                v_tile = vpool.tile([128, ELEM], mmdt)
                for tile_, cache in ((k_tile, kc_d), (v_tile, vc_d)):
                    nc.gpsimd.dma_gather(
                        tile_[:].rearrange("p (a f) -> p a f", a=1),
                        cache.ap().bitcast(mmdt),
                        idxs[:, t * 8 : (t + 1) * 8],
                        num_idxs=128,
                        num_idxs_reg=128,
                        elem_size=ELEM,
                        single_packet=False,
                    )
                mask_t = mpool.tile([16, 512], f32)
                nc.sync.dma_start(mask_t[:], mask_d.ap()[:, t * 512 : (t + 1) * 512])

                # K transposes: [tok, d] -> [d, tok] per kv head
                kt_sb = ktpool.tile([128, 8 * 512], mmdt)
                for h in range(KVH):
                    ktp = ktpsum.tile([128, 512], mmdt, tag="ktp")
                    for j in range(CHUNK):
                        nc.tensor.transpose(
                            ktp[:, j * 128 : (j + 1) * 128],
                            k_tile[:, (j * KVH + h) * 128 : (j * KVH + h + 1) * 128],
                            ident_r[:],
                        )
                    if h < 4:
                        nc.scalar.copy(kt_sb[:, h * 512 : (h + 1) * 512], ktp[:])
                    else:
                        nc.vector.tensor_copy(kt_sb[:, h * 512 : (h + 1) * 512], ktp[:])

                # per-head: scores (+mask) -> exp -> transpose into ptp cols
                ptp = ptpsum.tile([128, 512], f32, tag="ptp")
                for k in range(KVH):
                    sp = spsum.tile([16, 512], f32, tag="sp")
                    nc.tensor.matmul(
                        sp[:],
                        lhsT=qt_r[:, k * 16 : (k + 1) * 16],
                        rhs=kt_sb[:, k * 512 : (k + 1) * 512],
                        start=True,
                        stop=True,
                    )
                    scores = spool.tile([16, 512], f32, tag="scores")
                    nc.vector.tensor_tensor(
                        out=scores[:], in0=sp[:], in1=mask_t[:], op=Alu.add
                    )
                    probs = ppool.tile([16, 512], f32, tag="probs")
                    nc.scalar.activation(
                        probs[:], scores[:], Act.Exp,
                        accum_out=partials[:, t * 8 + k : t * 8 + k + 1],
                    )
                    for j in range(CHUNK):
                        nc.tensor.transpose(
                            ptp[:, j * 128 + k * 16 : j * 128 + (k + 1) * 16],
                            probs[:, j * 128 : (j + 1) * 128],
                            ident[0:16, 0:16],
                        )

                pt_sb = ptpool.tile([128, 512], mmdt)
                nc.vector.tensor_copy(pt_sb[:], ptp[:])

                # PV accumulate: out[(k,s,g), (k',d)] for k,k' in the g2 group
                for j in range(CHUNK):
                    for g2 in range(2):
                        nc.tensor.matmul(
                            pv[g2][:],
                            lhsT=pt_sb[:, j * 128 + g2 * 64 : j * 128 + g2 * 64 + 64],
                            rhs=rcv(v_tile[:, j * 1024 + g2 * 512 : j * 1024 + (g2 + 1) * 512]),
                            start=(rep == 0 and t == 0 and j == 0),
                            stop=(rep == repeat - 1 and t == T - 1
                                  and j == CHUNK - 1),
                            skip_group_check=True,
                        )

            # finalize: per-(s,g) sums per head -> 1/sum -> scale -> out
            sums = constp.tile([16, 8], f32)
            nc.vector.reduce_sum(
                out=sums[:],
                in_=partials[:].rearrange("p (t k) -> p k t", k=8),
                axis=mybir.AxisListType.X,
            )
            sums2 = constp.tile([16, 8], f32)
            nc.vector.tensor_scalar_max(sums2[:], sums[:], 1e-30)
            recip = constp.tile([16, 8], f32)
            nc.vector.reciprocal(recip[:], sums2[:])
            # rearrange recips to [64 rows (kl,s,g), 2 (g2)] via tiny DMAs
            recip2 = constp.tile([64, 2], f32)
            for k in range(KVH):
                g2, kl = divmod(k, 4)
                nc.sync.dma_start(
                    recip2[kl * 16 : (kl + 1) * 16, g2 : g2 + 1],
                    recip[:, k : k + 1],
                )

            pv_sb = constp.tile([64, 1024], f32)
            for g2 in range(2):
                nc.vector.tensor_copy(pv_sb[:, g2 * 512 : (g2 + 1) * 512], pv[g2][:])
            scaled = constp.tile([64, 1024], f32)
            for g2 in range(2):
                nc.vector.tensor_scalar(
                    out=scaled[:, g2 * 512 : (g2 + 1) * 512],
                    in0=pv_sb[:, g2 * 512 : (g2 + 1) * 512],
                    scalar1=recip2[:, g2 : g2 + 1],
                    scalar2=None,
                    op0=Alu.mult,
                )
            # out rows (k,s,g): 8 diagonal-block DMAs
            for k in range(KVH):
                g2, kl = divmod(k, 4)
                nc.sync.dma_start(
                    out_d.ap()[k * 16 : (k + 1) * 16, :],
                    scaled[kl * 16 : (kl + 1) * 16,
                           g2 * 512 + kl * 128 : g2 * 512 + (kl + 1) * 128],
                )

    nc.compile()
    return nc


_prog_cache = {}


def _get_program(T, use_f32r=True):
    key = (T, use_f32r)
    if key not in _prog_cache:
        _prog_cache[key] = _build_program(T, use_f32r)
    return _prog_cache[key]


def bench(q, k_new, v_new, k_cache, v_cache, block_tables, context_lens,
          slot_mapping, iters=5, use_f32r=False, repeat=1):
    """Stage inputs on device once, then time repeated executions."""
    import time

    import jax
    from jax.sharding import Mesh, NamedSharding, PartitionSpec
    from jax.experimental.shard_map import shard_map

    import concourse.bass2jax as b2j
    import concourse.mybir as mybir

    kc4, vc4, per_core, T, assign = _host_prepare(
        q, k_new, v_new, k_cache, v_cache, block_tables, context_lens
    )
    nc = _build_program(T, use_f32r=use_f32r, repeat=repeat)
    b2j.install_neuronx_cc_hook()

    in_maps = []
    for c in range(NCORES):
        pc = per_core[c]
        in_maps.append(
            {"kc4": kc4, "vc4": vc4, "qt": pc["qt"], "idx": pc["idx"],
             "mask": pc["mask"]}
        )

    pid_name = nc.partition_id_tensor.name if nc.partition_id_tensor else None
    in_names, out_names, out_avals, zero_outs = [], [], [], []
    for alloc in nc.m.functions[0].allocations:
        if not isinstance(alloc, mybir.MemoryLocationSet):
            continue
        name = alloc.memorylocations[0].name
        if alloc.kind == "ExternalInput":
            if name != pid_name:
                in_names.append(name)
        elif alloc.kind == "ExternalOutput":
            shape = tuple(alloc.tensor_shape)
            dtype = mybir.dt.np(alloc.dtype)
            out_names.append(name)
            out_avals.append(jax.core.ShapedArray(shape, dtype))
            zero_outs.append(np.zeros(shape, dtype))
    n_params = len(in_names)
    n_outs = len(out_avals)
    all_names = in_names + out_names
    if pid_name is not None:
        all_names = all_names + [pid_name]
    donate = tuple(range(n_params, n_params + n_outs))

    def _body(*args):
        operands = list(args)
        if pid_name is not None:
            operands.append(b2j.partition_id_tensor())
        outs = b2j._bass_exec_p.bind(
            *operands,
            out_avals=tuple(out_avals),
            in_names=tuple(all_names),
            out_names=tuple(out_names),
            lowering_input_output_aliases=(),
            sim_require_finite=True,
            sim_require_nnan=True,
            nc=nc,
        )
        return tuple(outs)

    devices = jax.devices()[:NCORES]
    mesh = Mesh(np.asarray(devices), ("core",))
    spec = NamedSharding(mesh, PartitionSpec("core"))
    sharded = jax.jit(
        shard_map(
            _body, mesh=mesh,
            in_specs=(PartitionSpec("core"),) * (n_params + n_outs),
            out_specs=(PartitionSpec("core"),) * n_outs,
            check_rep=False,
        ),
        donate_argnums=donate,
        keep_unused=True,
    )
    concat_in = [
        jax.device_put(
            np.concatenate([np.asarray(in_maps[c][n]) for c in range(NCORES)], 0),
            spec,
        )
        for n in in_names
    ]
    jax.block_until_ready(concat_in)

    def one_call():
        zeros = [
            jax.device_put(
                np.zeros((NCORES * z.shape[0], *z.shape[1:]), z.dtype), spec
            )
            for z in zero_outs
        ]
        jax.block_until_ready(zeros)
        t0 = time.perf_counter()
        out = sharded(*concat_in, *zeros)
        jax.block_until_ready(out)
        return time.perf_counter() - t0, out

    times = []
    out = None
    for _ in range(iters):
        dt, out = one_call()
        times.append(dt)

    res = [
        {n: np.asarray(out[i]).reshape(NCORES, *out_avals[i].shape)[c]
         for i, n in enumerate(out_names)}
        for c in range(NCORES)
    ]
    outv = np.zeros((B, 1, H, DH), np.float32)
    for c in range(NCORES):
        oc = res[c]["o"].reshape(KVH, SLOTS, G, DH)
        for slot, s in enumerate(per_core[c]["seqs"]):
            outv[s, 0] = oc[:, slot].reshape(H, DH)
    return times, outv


def kernel(q, k_new, v_new, k_cache, v_cache, block_tables, context_lens,
           slot_mapping, _trace=False):
    from concourse.bass_utils import run_bass_kernel_spmd

    kc4, vc4, per_core, T, assign = _host_prepare(
        q, k_new, v_new, k_cache, v_cache, block_tables, context_lens
    )
    nc = _get_program(T, use_f32r=False)

    in_maps = []
    for c in range(NCORES):
        pc = per_core[c]
        in_maps.append(
            {
                "kc4": kc4,
                "vc4": vc4,
                "qt": pc["qt"],
                "idx": pc["idx"],
                "mask": pc["mask"],
            }
        )
    res = run_bass_kernel_spmd(
        nc, in_maps, core_ids=list(range(NCORES)), trace=_trace
    )

    out = np.zeros((B, 1, H, DH), np.float32)
    for c in range(NCORES):
        oc = res.results[c]["o"].reshape(KVH, SLOTS, G, DH)
        for slot, s in enumerate(per_core[c]["seqs"]):
            out[s, 0] = oc[:, slot].reshape(H, DH)
    if _trace:
        kernel._last_results = res
    return out
